# revision 59
# baseline (speedup 1.0000x reference)
"""FILIP InfoNCE loss kernel for 8 Trainium2 NeuronCores.

Strategy (data-parallel over batch b=64, 8 batches per core):
  - Each core receives its t1 shard [8,196,256] plus the FULL t2, pre-rotated
    on host by np.roll(t2, -8c) so that every core's "diagonal" (matched-pair)
    batches are its local blocks 0..7.  This keeps the SPMD program identical
    across cores.
  - On-chip per core:
      * normalize tokens (1/||x|| via exact HW iterative divide), transpose to
        [d, token] layout with a diag-scaled identity matmul (transpose+scale
        fused on the PE).
      * big cross-similarity pass in bf16 (only feeds the loss scalar through
        mean/max -> logsumexp, tolerant to bf16; the per-(b1,b2) row-max bias
        is uniform and cancels in lse - diag).  The t2 column-group loop is
        OUTER so the cross pass streams behind t2 normalization.
      * segmented col-max over p2 on the Vector engine directly from PSUM.
      * per-b1 mean over p1 folded into a matmul with 1/||t1|| weights
        (t1 normalization commutes with max over p2).
      * logsumexp + diag extraction on-chip -> per-core partial loss [8].
      * exact-fp32 diagonal-block matmuls (both orientations) -> top-1
        argmax via DVE max/max_index -> idx12/idx21 (indices must match the
        fp32 reference exactly, so no bf16 here).
  - Host: sum the 64 partial losses / 64, concatenate index shards.
"""

import numpy as np

B_TOT = 64          # total batch
B_LOC = 8           # batches per core
P = 196             # patches per batch
D = 256             # embed dim
N1 = B_LOC * P      # 1568 tokens of t1 per core
N2 = B_TOT * P      # 12544 tokens of t2
N_CORES = 8
TEMPERATURE = 0.1

T1_TILES = (N1 + 127) // 128            # 13 (12 full + 32 rows)
T2_TILES = N2 // 128                    # 98, exact
NSEG = 392                              # matmul moving tile = 2 patch blocks
N_SEGS = N2 // NSEG                     # 32 segments total
JSEG = 3                                # segments per psum group (3 banks)
N_GROUPS = -(-N_SEGS // JSEG)           # 11 groups (10x3 + 1x2)
QUAD = 4                                # token tiles loaded per input DMA


def _build_program():
    import concourse.tile as tile
    from concourse import bacc, mybir
    from concourse.masks import make_identity

    fp32 = mybir.dt.float32
    bf16 = mybir.dt.bfloat16
    u32 = mybir.dt.uint32
    AF = mybir.ActivationFunctionType
    ALU = mybir.AluOpType
    AX = mybir.AxisListType

    nc = bacc.Bacc(None, target_bir_lowering=False)
    x1 = nc.declare_dram_parameter("x1", [N1, D], fp32, isOutput=False)
    x2 = nc.declare_dram_parameter("x2", [N2, D], fp32, isOutput=False)
    wmask_in = nc.declare_dram_parameter("wmask", [128, T1_TILES, B_LOC], fp32,
                                         isOutput=False)
    loss_out = nc.declare_dram_parameter("loss_part", [B_LOC, 1], fp32, isOutput=True)
    idx12_out = nc.declare_dram_parameter("idx12", [B_LOC, P], u32, isOutput=True)
    idx21_out = nc.declare_dram_parameter("idx21", [B_LOC, P], u32, isOutput=True)

    with tile.TileContext(nc) as tc:
        with (
            tc.tile_pool(name="big", bufs=1) as big,
            tc.tile_pool(name="work", bufs=8) as work,
            tc.tile_pool(name="dwork", bufs=4) as dwork,
            tc.tile_pool(name="pp", bufs=2, space="PSUM") as pp,
            tc.tile_pool(name="pd", bufs=2, space="PSUM") as pd,
        ):
            # persistent SBUF tensors
            ident_f = big.tile([128, 128], fp32, tag="ident_f")
            ident_b = big.tile([128, 128], bf16, tag="ident_b")
            make_identity(nc, ident_f)
            nc.gpsimd.tensor_copy(ident_b, ident_f)

            t1T_bf = big.tile([128, 2, T1_TILES * 128], bf16, tag="t1b")
            t1T_f = big.tile([128, 2, T1_TILES * 128], fp32, tag="t1f")
            t2T_f = big.tile([128, 2, T1_TILES * 128], fp32, tag="t2f")
            t2T_bf = big.tile([128, 2, N2], bf16, tag="t2b")
            rinv1 = big.tile([128, ((T1_TILES + QUAD - 1) // QUAD) * QUAD], fp32,
                             tag="rinv1")
            max12 = big.tile([128, T1_TILES, B_TOT], fp32, tag="max12")

            def load_quad(src_dram, src_n, q):
                """One DMA covering up to QUAD 128-token tiles."""
                t0 = q * QUAD
                rows_total = min(QUAD * 128, src_n - t0 * 128)
                ntile = (rows_total + 127) // 128
                raw = work.tile([128, QUAD, D], fp32, tag="raw",
                                name=f"rawq_{src_n}_{q}")
                src = src_dram[t0 * 128: t0 * 128 + rows_total, :]
                if q == 0:
                    # first quad: per-tile DMAs so the normalize chain starts
                    # as soon as the first 128 tokens land
                    for jj in range(ntile):
                        nc.sync.dma_start(
                            out=raw[:, jj, :],
                            in_=src_dram[(t0 + jj) * 128:(t0 + jj + 1) * 128, :])
                elif rows_total % 128 == 0:
                    nc.sync.dma_start(
                        out=raw[:, :ntile, :],
                        in_=src.rearrange("(a p) d -> p a d", p=128))
                else:
                    full = rows_total // 128
                    if full:
                        nc.sync.dma_start(
                            out=raw[:, :full, :],
                            in_=src_dram[t0 * 128: (t0 + full) * 128, :]
                            .rearrange("(a p) d -> p a d", p=128))
                    rem = rows_total - full * 128
                    nc.sync.dma_start(
                        out=raw[:rem, full, :],
                        in_=src_dram[(t0 + full) * 128: t0 * 128 + rows_total, :])
                return raw

            def stats_quad(raw_q, ntile, rinv_dst):
                """1/||row|| for up to QUAD tiles in one sqrt + one reciprocal.
                Stats run on all 128 rows even for a partial last tile
                (stale-but-finite data; masked out downstream) to keep
                per-instruction dependency fan-in low."""
                ssq = work.tile([128, QUAD], fp32, tag="ssq")
                for j in range(ntile):
                    sq = work.tile([128, D], fp32, tag="sq")
                    nc.scalar.activation(out=sq, in_=raw_q[:, j, :], func=AF.Square,
                                         accum_out=ssq[:, j:j + 1])
                nrm = work.tile([128, QUAD], fp32, tag="nrm")
                nc.scalar.sqrt(nrm[:, :ntile], ssq[:, :ntile])
                nc.vector.reciprocal(rinv_dst[:, :ntile], nrm[:, :ntile])

            def norm_tile(raw_q, j, t, rows, rinv_ap, bf_dst, f32_dst,
                          dve_copy=False):
                """Emit transposed normalized copies for one 128-token tile
                (slice j of a quad): bf16 always, exact-fp32 when f32_dst.
                dve_copy routes the PSUM->SBUF copies to the Vector engine
                (idle during the lead-in) instead of ScalarE."""
                raw = raw_q[:, j, :]
                dgf = None
                if f32_dst is not None:
                    dgf = work.tile([128, 128], fp32, tag="dgf")
                    nc.gpsimd.tensor_scalar(out=dgf[:rows, :rows],
                                            in0=ident_f[:rows, :rows],
                                            scalar1=rinv_ap[:rows], scalar2=None,
                                            op0=ALU.mult)
                if bf_dst is not None:
                    raw_b = work.tile([128, D], bf16, tag="raw_b")
                    nc.gpsimd.tensor_copy(raw_b[:rows], raw[:rows])
                    if bf_dst is t2T_bf:
                        dgb = work.tile([128, 128], bf16, tag="dgb")
                        if dgf is not None:
                            nc.gpsimd.tensor_copy(dgb[:rows, :rows], dgf[:rows, :rows])
                        else:
                            nc.gpsimd.tensor_scalar(out=dgb[:rows, :rows],
                                                    in0=ident_f[:rows, :rows],
                                                    scalar1=rinv_ap[:rows],
                                                    scalar2=None, op0=ALU.mult)
                        rhs_b = dgb
                    else:
                        rhs_b = ident_b
                    pt = pd.tile([128, 2, 128], fp32, tag="pd", name=f"ptb_{t}")
                    for k in range(2):
                        nc.tensor.matmul(out=pt[:, k, :rows],
                                         lhsT=raw_b[:rows, k * 128:(k + 1) * 128],
                                         rhs=rhs_b[:rows, :rows],
                                         start=True, stop=True)
                    if dve_copy:
                        nc.vector.tensor_copy(bf_dst[:, :, t * 128:t * 128 + rows],
                                              pt[:, :, :rows])
                    else:
                        nc.scalar.copy(out=bf_dst[:, :, t * 128:t * 128 + rows],
                                       in_=pt[:, :, :rows])
                if f32_dst is not None:
                    pt2 = pd.tile([128, 2, 128], fp32, tag="pd")
                    for k in range(2):
                        nc.tensor.matmul(out=pt2[:, k, :rows],
                                         lhsT=raw[:rows, k * 128:(k + 1) * 128],
                                         rhs=dgf[:rows, :rows], start=True, stop=True)
                    if dve_copy:
                        nc.vector.tensor_copy(f32_dst[:, :, t * 128:t * 128 + rows],
                                              pt2[:, :, :rows])
                    else:
                        nc.scalar.copy(out=f32_dst[:, :, t * 128:t * 128 + rows],
                                       in_=pt2[:, :, :rows])

            # t2 normalization is emitted INTERLEAVED with the cross pass so
            # every engine's instruction stream alternates between the two:
            # cross group g only needs t2 tiles < ceil((g+1)*NGRP/128), so we
            # emit norm tiles one group ahead, then the cross group.
            t2_done = 0          # t2 quads emitted so far

            def emit_t2_quad():
                nonlocal t2_done
                q = t2_done
                ntile = min(QUAD, T2_TILES - q * QUAD)
                raw_q = load_quad(x2, N2, q)
                rvq = work.tile([128, QUAD], fp32, tag="rv2")
                stats_quad(raw_q, ntile, rvq)
                for j in range(ntile):
                    t = q * QUAD + j
                    norm_tile(raw_q, j, t, 128, rvq[:, j:j + 1], t2T_bf,
                              t2T_f if t < T1_TILES else None,
                              dve_copy=(q < 3))
                t2_done += 1

            def emit_t2_norm_until(tile_needed):
                while t2_done * QUAD < min(tile_needed, T2_TILES):
                    emit_t2_quad()

            # t1 is needed in full as the cross stationary operand; interleave
            # its quads with the first t2 quads and with the first cross group
            # so the DVE reduce stream starts as early as possible.  t1's bf16
            # path stays RAW (1/||t1|| is folded into the mean weights); its
            # fp32 path is normalized (moving operand for idx21).
            def emit_t1_quad(q):
                raw_q = load_quad(x1, N1, q)
                ntile = min(QUAD, T1_TILES - q * QUAD)
                stats_quad(raw_q, ntile, rinv1[:, q * QUAD:q * QUAD + QUAD])
                for j in range(ntile):
                    t = q * QUAD + j
                    rows = min(128, N1 - t * 128)
                    norm_tile(raw_q, j, t, rows, rinv1[:, t:t + 1], t1T_bf, t1T_f,
                              dve_copy=(q < 2))

            def seg_end(g):
                return min(JSEG * (g + 1), N_SEGS)

            def tiles_for(g):
                # t2 tiles needed through the end of group g
                return -(-seg_end(g) * NSEG // 128)

            def emit_cross(g, m):
                mrows = min(128, N1 - m * 128)
                s0 = g * JSEG
                nseg = seg_end(g) - s0
                pc = pp.tile([128, JSEG, 512], fp32, tag="pp", name=f"pc_{g}_{m}")
                for j in range(nseg):
                    n0 = (s0 + j) * NSEG
                    for k in range(2):
                        nc.tensor.matmul(
                            out=pc[:mrows, j, 0:NSEG],
                            lhsT=t1T_bf[:, k, m * 128:m * 128 + mrows],
                            rhs=t2T_bf[:, k, n0:n0 + NSEG],
                            start=(k == 0), stop=(k == 1))
                nc.vector.tensor_reduce(
                    out=max12[:mrows, m, 2 * s0:2 * seg_end(g)],
                    in_=pc[:mrows, 0:nseg, 0:NSEG].rearrange("p j (s q) -> p j s q", q=P),
                    op=ALU.max, axis=AX.X)

            def emit_diag_block(i):
                # exact fp32 diagonal block i -> argmax indices; needs only the
                # first T1_TILES t2 tiles, and its PE/DVE/ACT work hides under
                # the cross pass
                base = i * P
                for orient, dst in ((0, idx12_out), (1, idx21_out)):
                    sta, mov = (t1T_f, t2T_f) if orient == 0 else (t2T_f, t1T_f)
                    for p0 in (0, 128):
                        pr = min(128, P - p0)
                        pblk = pd.tile([128, 256], fp32, tag="pd")
                        for k in range(2):
                            nc.tensor.matmul(
                                out=pblk[:pr, 0:P],
                                lhsT=sta[:, k, base + p0:base + p0 + pr],
                                rhs=mov[:, k, base:base + P],
                                start=(k == 0), stop=(k == 1))
                        sb = dwork.tile([128, P], fp32, tag="dsb")
                        nc.scalar.copy(out=sb[:pr, :], in_=pblk[:pr, 0:P])
                        mx = dwork.tile([128, 8], fp32, tag="dmx")
                        nc.vector.max(mx[:pr], sb[:pr, 0:P])
                        ix = dwork.tile([128, 8], u32, tag="dix")
                        nc.vector.max_index(ix[:pr], mx[:pr], sb[:pr, 0:P])
                        nc.sync.dma_start(out=dst[i, p0:p0 + pr],
                                          in_=ix[:pr, 0:1])

            t1_quads = (T1_TILES + QUAD - 1) // QUAD
            emit_t1_quad(0)
            emit_t2_quad()
            emit_t2_quad()
            emit_t2_quad()           # t2 tiles 0..11 cover cross group 0
            for m in range(0, QUAD):
                emit_cross(0, m)
            emit_t1_quad(1)
            for q in range(2, t1_quads):
                emit_t1_quad(q)
                emit_t2_quad()
                for m in range((q - 1) * QUAD, min(q * QUAD, T1_TILES)):
                    emit_cross(0, m)
            emit_t2_quad()
            for m in range((t1_quads - 1) * QUAD, T1_TILES):
                emit_cross(0, m)

            # zero-padded per-(m, b1) weight columns for the p1-mean:
            # PE operands must start at partition base 0/32/64, so multiply a
            # constant host-provided 0/(1/(P*T)) membership mask by rinv1.
            wm = big.tile([128, T1_TILES, B_LOC], fp32, tag="wm")
            nc.sync.dma_start(out=wm[:, :, :], in_=wmask_in[:, :, :])
            wseg = big.tile([128, T1_TILES, B_LOC], fp32, tag="wseg")
            for m in range(T1_TILES):
                nc.gpsimd.tensor_scalar(out=wseg[:, m, :], in0=wm[:, m, :],
                                        scalar1=rinv1[:, m:m + 1], scalar2=None,
                                        op0=ALU.mult)
            psim = pd.tile([B_LOC, B_TOT], fp32, tag="pd")

            for g in range(1, N_GROUPS):
                need = tiles_for(g + 2)                 # two-group lookahead
                for m in range(T1_TILES):
                    emit_cross(g, m)
                    if m % 2 == 1 and t2_done * QUAD < min(need, T2_TILES):
                        emit_t2_quad()
                    if g == N_GROUPS - 1:
                        # fold the p1-mean accumulation into the last group so
                        # only the final loss chain remains as a tail
                        mrows = min(128, N1 - m * 128)
                        nc.tensor.matmul(out=psim[0:B_LOC, :],
                                         lhsT=wseg[0:mrows, m, 0:B_LOC],
                                         rhs=max12[0:mrows, m, 0:B_TOT],
                                         start=(m == 0), stop=(m == T1_TILES - 1))
                emit_t2_norm_until(need)
                if 3 <= g <= 10:
                    # t2T_f (first T1_TILES tiles) is complete by then
                    emit_diag_block(g - 3)

            # loss: mean_b1( lse(logits_row) - logits[b1, b1] ); the 1/(P*T)
            # logits scale is folded into wmask on the host
            logits = dwork.tile([B_LOC, B_TOT], fp32, tag="logits")
            nc.scalar.copy(logits, psim[0:B_LOC, :])
            rmax = dwork.tile([B_LOC, 1], fp32, tag="rmax")
            nc.vector.tensor_reduce(out=rmax, in_=logits, op=ALU.max, axis=AX.X)
            nmax = dwork.tile([B_LOC, 1], fp32, tag="nmax")
            nc.vector.tensor_scalar_mul(nmax, rmax, -1.0)
            expt = dwork.tile([B_LOC, B_TOT], fp32, tag="expt")
            sume = dwork.tile([B_LOC, 1], fp32, tag="sume")
            nc.scalar.activation(out=expt, in_=logits, func=AF.Exp,
                                 bias=nmax, scale=1.0, accum_out=sume)
            lse = dwork.tile([B_LOC, 1], fp32, tag="lse")
            nc.scalar.activation(out=lse, in_=sume, func=AF.Ln)
            dprod = dwork.tile([B_LOC, B_TOT], fp32, tag="dprod")
            dsum = dwork.tile([B_LOC, 1], fp32, tag="dsum")
            nc.vector.tensor_tensor(out=dprod, in0=logits, in1=ident_f[0:B_LOC, 0:B_TOT],
                                    op=ALU.mult)
            nc.vector.tensor_reduce(out=dsum, in_=dprod, op=ALU.add, axis=AX.X)
            part = dwork.tile([B_LOC, 1], fp32, tag="part")
            nc.vector.tensor_tensor(out=part, in0=lse, in1=rmax, op=ALU.add)
            nc.vector.tensor_tensor(out=part, in0=part, in1=dsum, op=ALU.subtract)
            nc.sync.dma_start(out=loss_out[:, :], in_=part[0:B_LOC, 0:1])
    return nc


_CACHED = {}


def _get_program():
    if "nc" not in _CACHED:
        nc = _build_program()
        if not nc.is_finalized():
            nc.finalize()   # Bacc: runs wait-legalization + cleanup passes
        _CACHED["nc"] = nc
    return _CACHED["nc"]


def kernel(batch_tokens1, batch_tokens2, labels=None, epoch=None, _trace=False):
    from concourse.bass_utils import run_bass_kernel_spmd

    t1 = np.ascontiguousarray(np.asarray(batch_tokens1, dtype=np.float32))
    t2 = np.ascontiguousarray(np.asarray(batch_tokens2, dtype=np.float32))
    assert t1.shape == (B_TOT, P, D) and t2.shape == (B_TOT, P, D)

    # constant membership mask: wmask[r, m, b1] = 1/(P*T) when global t1 token
    # 128*m + r belongs to local batch b1, else 0  (logits scale folded in)
    wmask = np.zeros((128, T1_TILES, B_LOC), dtype=np.float32)
    for m in range(T1_TILES):
        for r in range(min(128, N1 - m * 128)):
            b1 = (m * 128 + r) // P
            wmask[r, m, b1] = 1.0 / (P * TEMPERATURE)

    in_maps = []
    for c in range(N_CORES):
        x1 = t1[c * B_LOC:(c + 1) * B_LOC].reshape(N1, D)
        x2 = np.roll(t2, -c * B_LOC, axis=0).reshape(N2, D)
        in_maps.append({"x1": np.ascontiguousarray(x1),
                        "x2": np.ascontiguousarray(x2),
                        "wmask": wmask})

    nc = _get_program()
    if _trace:
        try:
            out = run_bass_kernel_spmd(nc, in_maps, list(range(N_CORES)), trace=True)
            _CACHED["exec_time_ns"] = out.exec_time_ns
            _CACHED["bkr"] = out
        except Exception:
            # NTFF profiling hook unavailable (e.g. minimal axon client):
            # fall back to an untraced run
            out = run_bass_kernel_spmd(nc, in_maps, list(range(N_CORES)))
    else:
        out = run_bass_kernel_spmd(nc, in_maps, list(range(N_CORES)))
    res = out.results

    loss = np.float32(sum(float(r["loss_part"].sum()) for r in res) / B_TOT)
    idx12 = np.concatenate([r["idx12"].astype(np.int32) for r in res], axis=0)
    idx21 = np.concatenate([r["idx21"].astype(np.int32) for r in res], axis=0)
    return loss, idx12, idx21
